# revision 67
# baseline (speedup 1.0000x reference)
"""OnlineTripletLoss Trainium2 kernel (8 NeuronCores, SPMD).

Strategy (label-space mining):
  pos_mask = positive_idxs[:, target_idx] is a column permutation of the raw
  mask. Instead of permuting the 16MB masks, permute the 2MB embedding once:
  g[l] = embedding[inv_target[l]].  Mining for anchor i then runs over label
  axis l with the raw (contiguous) masks:
      d2'[i,l] = C0 + ||e_i - g_l + eps||^2   (expanded, via PE matmul)
      hardest pos: max over l of 2*d2'[i,l] * mp[i,l]       (mp in {0,1})
      hardest neg: min over l of 2*d2'[i,l] * wn[i,l]       (wn in {1,200})
  Both minings run as ONE custom DVE pass per side over the same staged
  tensor (PACK_IDX_RMAX_ANT / PACK_IDX_RMIN_ANT):
  the op rounds the masked value to an integer (2^23 trick), packs
  (value, label index) into a single exactly-representable f32
  (value*4096 + idx <= 2^24) and max-reduces it, so the winning label
  index falls out of the accum's low mantissa bits -- no separate
  max_index scan. p/n rows are then gathered by indirect DMA and
  ap/an/pn recomputed exactly in f32 (avoids both the quantization and
  the winner's-curse bias of the mined values).

Per core: 512 anchors x 4096 labels, 4 blocks of 128 anchors.
Outputs per core: masked per-anchor loss and validity; host sums and divides.
"""

import numpy as np
import ml_dtypes

import concourse.bass as bass
import concourse.mybir as mybir
import concourse.tile as tile
from concourse import bacc
from concourse.bass_utils import run_bass_kernel_spmd
import concourse.dve_ops as dve_ops
from concourse.dve_ops import DveOp
from concourse.dve_spec import (Spec, Src0, Src1, Idx, Zero, MaxNeg, maxx,
                                minn, lower, _has_src1)
from concourse.dve_spec import C0 as DVE_C0, C1 as DVE_C1, C2 as DVE_C2
from concourse.dve_uop import DveOpSpec

_OPNAME = "PACK_IDX_RMAX_ANT"


def _ref_pack_idx_rmax(in0, in1, s0, s1, imm2):
    t = (in0.astype(np.float32) * in1).astype(np.float32)
    q = np.float32(np.float32(t + s1) - s1)          # round-to-nearest int
    q = q.reshape(q.shape[0], -1)
    p = np.float32(q * np.float32(imm2)
                   + np.arange(q.shape[-1], dtype=np.float32)[None, :])
    mx = np.maximum(np.float32(0.0), p.max(axis=-1, keepdims=True))
    return p.reshape(in0.shape), mx.astype(np.float32)


def register_pack_idx_rmax():
    """Custom DVE op: p[k] = round((in0[k]+s0)*in1[k])*imm2 + k,
    accum_out = max(0, row-max(p)).

    One pass fuses the mask multiply, integer quantization (s1 = 2^23
    round trick), and packing of (quantized value, element index) into one
    exactly-representable f32 (imm2 = 4096 shift), max-reduced. The winning
    label index comes out of the accum's low mantissa bits -- no separate
    max_index scan is needed. in0 is pre-biased by ACT (pos: 2*d2, neg:
    K - 2*d2 via scale=-1), masks are {0,1}. s0 is unused.
    """
    if _OPNAME in dve_ops._SUB_OPCODE_FOR_NAME:
        for op in dve_ops.OPS:
            if op.name == _OPNAME:
                return op
    spec = Spec(body=((Src0 * Src1 + DVE_C1) - DVE_C1) * DVE_C2 + Idx,
                accum=maxx, accum_init=Zero, reference=_ref_pack_idx_rmax)
    row = max(dve_ops._SUB_OPCODE_FOR_NAME.values()) + 1
    assert row < 0x20
    shas = {}
    for ver in ("v3", "v4"):
        try:
            s = DveOpSpec(name=_OPNAME, opcode=row, uops=lower(spec, ver=ver),
                          rd1_en=_has_src1(spec))
            shas[ver] = s.sha(ver)
        except Exception:
            pass
    op = DveOp(_OPNAME, spec, subdim=False, uops_sha=shas)
    dve_ops.OPS.append(op)
    dve_ops.CUSTOM_DVE_SPECS[_OPNAME] = spec
    dve_ops._SUB_OPCODE_FOR_NAME[_OPNAME] = row
    return op


register_tt_mul_rmax = register_pack_idx_rmax  # back-compat alias

_OPNAME_MIN = "PACK_IDX_RMIN_ANT"


def _ref_pack_idx_rmin(in0, in1, s0, s1, imm2):
    t = (in0.astype(np.float32) * in1).astype(np.float32)
    q = np.float32(np.float32(t + s1) - s1)
    q = q.reshape(q.shape[0], -1)
    p = np.float32(np.float32(s1) - np.float32(
        q * np.float32(imm2)
        + np.arange(q.shape[-1], dtype=np.float32)[None, :]))
    mx = np.maximum(np.float32(0.0), p.max(axis=-1, keepdims=True))
    return p.reshape(in0.shape), mx.astype(np.float32)


def register_pack_idx_rmin():
    """Min twin of PACK_IDX_RMAX_ANT via inverted packing:
    p[k] = 2^23 - (round(in0*in1)*imm2 + k), accum = max(0, row-max(p))
    = 2^23 - min(packed). Valid candidates stay positive so the free
    Zero accum-init is safe (MaxNeg/C0 inits exceed the 6 carry lanes);
    invalid {weight 200} entries go hugely negative and lose.

    Mines the hardest negative off the SAME staged tensor as the max op:
    the {1,200} weight mask keeps invalid entries out of the min (200*d2
    overflows the exact-packing range, but losers only need to be large,
    not exact), so no second ACT staging pass is needed.
    """
    if _OPNAME_MIN in dve_ops._SUB_OPCODE_FOR_NAME:
        for op in dve_ops.OPS:
            if op.name == _OPNAME_MIN:
                return op
    spec = Spec(body=DVE_C1 - (((Src0 * Src1 + DVE_C1) - DVE_C1) * DVE_C2 + Idx),
                accum=maxx, accum_init=Zero, reference=_ref_pack_idx_rmin)
    row = max(dve_ops._SUB_OPCODE_FOR_NAME.values()) + 1
    assert row < 0x20
    shas = {}
    for ver in ("v3", "v4"):
        try:
            s = DveOpSpec(name=_OPNAME_MIN, opcode=row, uops=lower(spec, ver=ver),
                          rd1_en=_has_src1(spec))
            shas[ver] = s.sha(ver)
        except Exception:
            pass
    op = DveOp(_OPNAME_MIN, spec, subdim=False, uops_sha=shas)
    dve_ops.OPS.append(op)
    dve_ops.CUSTOM_DVE_SPECS[_OPNAME_MIN] = spec
    dve_ops._SUB_OPCODE_FOR_NAME[_OPNAME_MIN] = row
    return op

B, D = 4096, 128
M = 8              # cores
BL = B // M        # 512 anchors per core
P = 128            # partition block
NB = BL // P       # 4 anchor blocks per core
CH = 512           # psum chunk (one bank of f32)
NCH = B // CH      # 8 chunks
EPS = 1e-6
C0 = 32.0
MARGIN = 1.0

F32 = mybir.dt.float32
BF16 = mybir.dt.bfloat16
U8 = mybir.dt.uint8
I8 = mybir.dt.int8
U32 = mybir.dt.uint32
TWO23 = float(2.0 ** 23)
PACK = 4096.0
NEG_K = 2000.0
VTH_P = 32.0 * PACK        # packed validity threshold, pos
VTH_N = 1.0e6              # packed validity threshold, neg (inverted-min side)


def build_nc(debug: bool = False):
    pack_op = register_pack_idx_rmax()
    pack_min = register_pack_idx_rmin()
    nc = bacc.Bacc("TRN2", target_bir_lowering=False, debug=debug)

    eT = nc.dram_tensor("eT", [P, BL], BF16, kind="ExternalInput")      # -4*e_local^T
    gT = nc.dram_tensor("gT", [P, B], BF16, kind="ExternalInput")       # g^T
    cgo = nc.dram_tensor("cgo", [2, B], BF16, kind="ExternalInput")     # [2*cg ; ones]
    ar2 = nc.dram_tensor("ar2", [2, BL], BF16, kind="ExternalInput")    # [ones ; 2*arow]
    el = nc.dram_tensor("el", [P, NB, D], F32, kind="ExternalInput")    # anchor rows f32
    gfull = nc.dram_tensor("gfull", [B, D], F32, kind="ExternalInput")  # gather source
    mp = nc.dram_tensor("mp", [BL, B], U8, kind="ExternalInput")        # pos mask {0,1}
    wn = nc.dram_tensor("wn", [BL, B], U8, kind="ExternalInput")        # neg weights {1,200}

    lossv = nc.dram_tensor("lossv", [P, NB], F32, kind="ExternalOutput")
    vout = nc.dram_tensor("vout", [P, NB], F32, kind="ExternalOutput")

    with tile.TileContext(nc) as tc:
        with (
            tc.tile_pool(name="singles", bufs=1) as singles,
            tc.tile_pool(name="masks", bufs=3) as maskpool,
            tc.tile_pool(name="vscr", bufs=1) as vpool,
            tc.tile_pool(name="stage", bufs=2) as stagepool,
            tc.tile_pool(name="psum", bufs=2, space="PSUM") as psumpool,
            tc.tile_pool(name="sm", bufs=1) as sm,
            tc.tile_pool(name="blk", bufs=2) as blk,
        ):
            HB = B // 2          # 2048: one PSUM half (4 banks)
            HCH = HB // CH       # 4 chunks per half
            HB0 = HB             # first-mask split point
            # block-sliced eT and chunk-sliced gT loads so the first matmul
            # only waits on the slices it actually reads; block-0 masks are
            # interleaved after gT's first half so they land before mining
            eT_s = singles.tile([P, BL], BF16)
            nc.sync.dma_start(eT_s[:, 0:P], eT[:, 0:P])
            gT_s = singles.tile([P, B], BF16)
            for c in range(NCH):
                cs = slice(c * CH, (c + 1) * CH)
                nc.sync.dma_start(gT_s[:, cs], gT[:, cs])
            for b in range(1, NB):
                bs = slice(b * P, (b + 1) * P)
                nc.sync.dma_start(eT_s[:, bs], eT[:, bs])
            cgo_s = singles.tile([2, B], BF16)
            nc.sync.dma_start(cgo_s[:], cgo[:])
            ar2_s = singles.tile([2, BL], BF16)
            nc.sync.dma_start(ar2_s[:], ar2[:])
            eps_b = singles.tile([P, 1], F32)
            nc.vector.memset(eps_b[:], EPS)
            # touch Sqrt/Square/Relu once so ACT's table swap lands in the
            # fill shadow instead of the tail
            warm = singles.tile([P, 1], F32)
            nc.scalar.activation(warm[:], eps_b[:],
                                 mybir.ActivationFunctionType.Square)
            nc.scalar.activation(warm[:], warm[:],
                                 mybir.ActivationFunctionType.Sqrt)
            nc.scalar.activation(warm[:], warm[:],
                                 mybir.ActivationFunctionType.Relu)

            # batched per-anchor state (host pre-arranged contiguous):
            el_all = singles.tile([P, NB, D], F32)
            Mp_all = singles.tile([P, NB], F32)
            Mn_all = singles.tile([P, NB], F32)
            idxp_all = singles.tile([P, NB], U32)
            idxn_all = singles.tile([P, NB], U32)
            p_all = singles.tile([P, NB, D], F32)
            n_all = singles.tile([P, NB, D], F32)
            m4095 = singles.tile([P, 1], U32)
            nc.vector.memset(m4095[:], 4095)

            for b in range(NB):
                rs = b * P
                mp_b = maskpool.tile([P, B], U8, tag="mp")
                # block 0 masks ride the sync queue (needed first); later
                # blocks go via the mostly-idle GpSimd queue so the sync
                # queue drains gT/eT faster during the fill
                eng = nc.sync if b == 0 else nc.gpsimd
                eng.dma_start(mp_b[:], mp[rs:rs + P, :])
                wn_b = maskpool.tile([P, B], U8, tag="wn")
                eng.dma_start(wn_b[:], wn[rs:rs + P, :])
                if b == 1:
                    # tail-only data, loaded once the startup rush is over
                    nc.scalar.dma_start(el_all[:], el[:])

                v = vpool.tile([P, B], F32)
                # block 0 splits the first half into small pieces so the first
                # mining op starts as soon as one 512-col PSUM bank is ready
                pieces = [(0, HB), (HB, HB)]
                NPC = len(pieces)
                Ph = blk.tile([P, 2 * 3], F32, tag="Ph")  # [pos x pieces|neg x pieces]
                for pi, (hs, plen) in enumerate(pieces):
                    # grouped by lhsT so LDWEIGHTS isn't reloaded per chunk
                    psum = psumpool.tile([P, HB], F32, tag="psum")
                    pch = plen // CH
                    for c in range(pch):
                        cs = slice(hs + c * CH, hs + (c + 1) * CH)
                        ps = slice(c * CH, (c + 1) * CH)
                        nc.tensor.matmul(
                            psum[:, ps], lhsT=eT_s[:, rs:rs + P],
                            rhs=gT_s[:, cs], start=True, stop=False,
                        )
                    for c in range(pch):
                        cs = slice(hs + c * CH, hs + (c + 1) * CH)
                        ps = slice(c * CH, (c + 1) * CH)
                        nc.tensor.matmul(
                            psum[:, ps], lhsT=ar2_s[:, rs:rs + P],
                            rhs=cgo_s[:, cs], start=False, stop=True,
                        )

                    # ACT copies PSUM to SBUF so the PSUM slot is released
                    # after ~2us for PE's next half (DVE holding PSUM through
                    # both packs serializes PE and costs ~4us/block)
                    hsl = slice(hs, hs + plen)
                    dps = stagepool.tile([P, HB], F32, tag="dps")
                    nc.scalar.activation(
                        dps[:, 0:plen], psum[:, 0:plen],
                        mybir.ActivationFunctionType.Copy)

                    # packed mining (per-anchor bias rode the K=2 rank-1
                    # matmul): one DVE pass per side yields
                    # max(round(value)*4096 + local_idx) per partition
                    nc.vector._custom_dve(
                        pack_op, out=v[:, hsl], in0=dps[:, 0:plen],
                        in1=mp_b[:, hsl], s0=0.0, s1=TWO23, imm2=PACK,
                        accum_out=Ph[:, pi:pi + 1])
                    nc.vector._custom_dve(
                        pack_min, out=v[:, hsl], in0=dps[:, 0:plen],
                        in1=wn_b[:, hsl], s0=0.0, s1=TWO23, imm2=PACK,
                        accum_out=Ph[:, 3 + pi:4 + pi])

                # merge pieces (local idx -> global: +piece offset in idx field)
                for pi, (hs, plen) in enumerate(pieces):
                    if hs == 0:
                        continue
                    nc.vector.tensor_scalar(
                        Ph[:, pi:pi + 1], Ph[:, pi:pi + 1], float(hs),
                        scalar2=None, op0=mybir.AluOpType.add)
                    nc.vector.tensor_scalar(
                        Ph[:, 3 + pi:4 + pi], Ph[:, 3 + pi:4 + pi], float(-hs),
                        scalar2=None, op0=mybir.AluOpType.add)
                nc.vector.tensor_tensor(out=Mp_all[:, b:b + 1], in0=Ph[:, 0:1],
                                        in1=Ph[:, 1:2], op=mybir.AluOpType.max)
                nc.vector.tensor_tensor(out=Mn_all[:, b:b + 1], in0=Ph[:, 3:4],
                                        in1=Ph[:, 4:5], op=mybir.AluOpType.max)
                if NPC > 2:
                    nc.vector.tensor_tensor(
                        out=Mp_all[:, b:b + 1], in0=Mp_all[:, b:b + 1],
                        in1=Ph[:, 2:3], op=mybir.AluOpType.max)
                    nc.vector.tensor_tensor(
                        out=Mn_all[:, b:b + 1], in0=Mn_all[:, b:b + 1],
                        in1=Ph[:, 5:6], op=mybir.AluOpType.max)
                # decode indices: +2^23 pins ulp=1, low 12 mantissa bits = idx
                p23 = blk.tile([P, 2], F32, tag="p23")
                nc.vector.tensor_scalar(p23[:, 0:1], Mp_all[:, b:b + 1], TWO23,
                                        scalar2=None, op0=mybir.AluOpType.add)
                nc.vector.tensor_scalar(p23[:, 1:2], Mn_all[:, b:b + 1], -1.0,
                                        scalar2=None, op0=mybir.AluOpType.mult)
                nc.vector.tensor_scalar(p23[:, 1:2], p23[:, 1:2], 2.0 * TWO23,
                                        scalar2=None, op0=mybir.AluOpType.add)
                nc.vector.tensor_tensor(
                    out=idxp_all[:, b:b + 1], in0=p23[:, 0:1].bitcast(U32),
                    in1=m4095[:, 0:1], op=mybir.AluOpType.bitwise_and)
                nc.vector.tensor_tensor(
                    out=idxn_all[:, b:b + 1], in0=p23[:, 1:2].bitcast(U32),
                    in1=m4095[:, 0:1], op=mybir.AluOpType.bitwise_and)

                nc.gpsimd.indirect_dma_start(
                    out=p_all[:, b, :], out_offset=None, in_=gfull[:],
                    in_offset=bass.IndirectOffsetOnAxis(
                        ap=idxp_all[:, b:b + 1], axis=0),
                )
                nc.gpsimd.indirect_dma_start(
                    out=n_all[:, b, :], out_offset=None, in_=gfull[:],
                    in_offset=bass.IndirectOffsetOnAxis(
                        ap=idxn_all[:, b:b + 1], axis=0),
                )

            # ---- batched tail ----
            # exact f32: ap=||a-p+eps||, an=||a-n+eps||, pn=||p-n+eps||
            # split: blocks [0, NB-1) first (their gathers are long done while
            # block NB-1's gathers are still in flight), then the last block
            # validity first: depends only on the TTR accums, fills the DVE
            # stream while the last block's gathers are in flight
            vp = sm.tile([P, NB], F32)
            vn = sm.tile([P, NB], F32)
            valid = sm.tile([P, NB], F32)
            nc.vector.tensor_scalar(vp[:], Mp_all[:], VTH_P, scalar2=None,
                                    op0=mybir.AluOpType.is_gt)
            nc.vector.tensor_scalar(vn[:], Mn_all[:], VTH_N, scalar2=None,
                                    op0=mybir.AluOpType.is_gt)
            nc.vector.tensor_mul(valid[:], vp[:], vn[:])

            dif = sm.tile([P, NB, D], F32)
            sq = sm.tile([P, NB, D], F32)
            rt2 = sm.tile([P, 3 * NB], F32)   # [ap2 x NB | an2 x NB | pn2 x NB]
            pairs = ((el_all, p_all), (el_all, n_all), (p_all, n_all))
            for lo, hi in ((0, NB - 1), (NB - 1, NB)):
                n = hi - lo
                for k, (x, y) in enumerate(pairs):
                    nc.vector.tensor_sub(dif[:, lo:hi, :], x[:, lo:hi, :],
                                         y[:, lo:hi, :])
                    nc.scalar.activation(sq[:, lo:hi, :], dif[:, lo:hi, :],
                                         mybir.ActivationFunctionType.Square,
                                         bias=eps_b[:, 0:1], scale=1.0)
                    nc.vector.tensor_reduce(
                        out=rt2[:, k * NB + lo:k * NB + hi],
                        in_=sq[:, lo:hi, :],
                        axis=mybir.AxisListType.X, op=mybir.AluOpType.add)
            rt = sm.tile([P, 3 * NB], F32)
            nc.scalar.activation(rt[:], rt2[:], mybir.ActivationFunctionType.Sqrt)

            mn2 = sm.tile([P, NB], F32)
            nc.vector.tensor_tensor(out=mn2[:], in0=rt[:, NB:2 * NB],
                                    in1=rt[:, 2 * NB:3 * NB],
                                    op=mybir.AluOpType.min)
            dff = sm.tile([P, NB], F32)
            nc.vector.tensor_sub(dff[:], rt[:, 0:NB], mn2[:])
            lossb = sm.tile([P, NB], F32)
            nc.scalar.activation(lossb[:], dff[:],
                                 mybir.ActivationFunctionType.Relu,
                                 bias=MARGIN, scale=1.0)
            lout = sm.tile([P, NB], F32)
            nc.vector.tensor_mul(lout[:], lossb[:], valid[:])

            nc.sync.dma_start(lossv[:], lout[:])
            nc.sync.dma_start(vout[:], valid[:])

    nc.finalize()
    return nc


def make_in_maps(embedding, target_idx, positive_idxs, negative_idxs):
    e = np.asarray(embedding, np.float32)
    tid = np.asarray(target_idx, np.int64)
    pos = np.asarray(positive_idxs)
    neg = np.asarray(negative_idxs)

    inv = np.empty(B, np.int64)
    inv[tid] = np.arange(B)
    g = np.ascontiguousarray(e[inv])                       # [B, D] f32

    e64 = e.astype(np.float64)
    g64 = g.astype(np.float64)
    sq_a = (e64 * e64).sum(1)
    s_a = e64.sum(1)
    sq_g = (g64 * g64).sum(1)
    s_g = g64.sum(1)

    gT_bf = np.ascontiguousarray(g.T).astype(ml_dtypes.bfloat16)         # [D, B]
    # doubled pipeline so packed quantization is 0.5 d2-units
    cgo_np = np.ones((2, B), np.float32)
    cgo_np[0] = 2.0 * (sq_g - 2.0 * EPS * s_g)
    cgo_bf = cgo_np.astype(ml_dtypes.bfloat16)
    arow_full = np.asarray(
        2.0 * (sq_a + 2.0 * EPS * s_a + D * EPS * EPS + C0), np.float32)

    in_maps = []
    for m in range(M):
        r = slice(m * BL, (m + 1) * BL)
        # [P, NB(, D)] layouts: block index on the free axis
        el3 = np.ascontiguousarray(
            e[r].reshape(NB, P, D).transpose(1, 0, 2))
        ar2_np = np.ones((2, BL), np.float32)
        ar2_np[1] = arow_full[r]
        in_maps.append({
            "eT": np.ascontiguousarray((-4.0 * e[r].T)).astype(ml_dtypes.bfloat16),
            "gT": gT_bf,
            "cgo": cgo_bf,
            "ar2": ar2_np.astype(ml_dtypes.bfloat16),
            "el": el3,
            "gfull": g,
            "mp": np.ascontiguousarray(pos[r].astype(np.uint8)),
            "wn": np.ascontiguousarray(np.where(neg[r], 1, 200).astype(np.uint8)),
        })
    return in_maps


_NC_CACHE = {}


def kernel(embedding, target_idx, positive_idxs, negative_idxs):
    in_maps = make_in_maps(embedding, target_idx, positive_idxs, negative_idxs)
    if "nc" not in _NC_CACHE:
        _NC_CACHE["nc"] = build_nc(debug=False)
    nc = _NC_CACHE["nc"]
    res = run_bass_kernel_spmd(nc, in_maps, core_ids=list(range(M)))
    total_loss = np.float64(0.0)
    total_valid = np.float64(0.0)
    for r in res.results:
        total_loss += np.asarray(r["lossv"], np.float64).sum()
        total_valid += np.asarray(r["vout"], np.float64).sum()
    return np.float32(total_loss / max(total_valid, 1.0))


# revision 68
# speedup vs baseline: 1.0399x; 1.0399x over previous
"""OnlineTripletLoss Trainium2 kernel (8 NeuronCores, SPMD).

Strategy (label-space mining):
  pos_mask = positive_idxs[:, target_idx] is a column permutation of the raw
  mask. Instead of permuting the 16MB masks, permute the 2MB embedding once:
  g[l] = embedding[inv_target[l]].  Mining for anchor i then runs over label
  axis l with the raw (contiguous) masks:
      d2'[i,l] = C0 + ||e_i - g_l + eps||^2   (expanded, via PE matmul)
      hardest pos: max over l of 2*d2'[i,l] * mp[i,l]       (mp in {0,1})
      hardest neg: min over l of 2*d2'[i,l] * wn[i,l]       (wn in {1,200})
  Both minings run as ONE custom DVE pass per side over the same staged
  tensor (PACK_IDX_RMAX_ANT / PACK_IDX_RMIN_ANT):
  the op rounds the masked value to an integer (2^23 trick), packs
  (value, label index) into a single exactly-representable f32
  (value*4096 + idx <= 2^24) and max-reduces it, so the winning label
  index falls out of the accum's low mantissa bits -- no separate
  max_index scan. p/n rows are then gathered by indirect DMA and
  ap/an/pn recomputed exactly in f32 (avoids both the quantization and
  the winner's-curse bias of the mined values).

Per core: 512 anchors x 4096 labels, 4 blocks of 128 anchors.
Outputs per core: masked per-anchor loss and validity; host sums and divides.
"""

import numpy as np
import ml_dtypes

import concourse.bass as bass
import concourse.mybir as mybir
import concourse.tile as tile
from concourse import bacc
from concourse.bass_utils import run_bass_kernel_spmd
import concourse.dve_ops as dve_ops
from concourse.dve_ops import DveOp
from concourse.dve_spec import (Spec, Src0, Src1, Idx, Zero, MaxNeg, maxx,
                                minn, lower, _has_src1)
from concourse.dve_spec import C0 as DVE_C0, C1 as DVE_C1, C2 as DVE_C2
from concourse.dve_uop import DveOpSpec

_OPNAME = "PACK_IDX_RMAX_ANT"


def _ref_pack_idx_rmax(in0, in1, s0, s1, imm2):
    t = (in0.astype(np.float32) * in1).astype(np.float32)
    q = np.float32(np.float32(t + s1) - s1)          # round-to-nearest int
    q = q.reshape(q.shape[0], -1)
    p = np.float32(q * np.float32(imm2)
                   + np.arange(q.shape[-1], dtype=np.float32)[None, :])
    mx = np.maximum(np.float32(0.0), p.max(axis=-1, keepdims=True))
    return p.reshape(in0.shape), mx.astype(np.float32)


def register_pack_idx_rmax():
    """Custom DVE op: p[k] = round((in0[k]+s0)*in1[k])*imm2 + k,
    accum_out = max(0, row-max(p)).

    One pass fuses the mask multiply, integer quantization (s1 = 2^23
    round trick), and packing of (quantized value, element index) into one
    exactly-representable f32 (imm2 = 4096 shift), max-reduced. The winning
    label index comes out of the accum's low mantissa bits -- no separate
    max_index scan is needed. in0 is pre-biased by ACT (pos: 2*d2, neg:
    K - 2*d2 via scale=-1), masks are {0,1}. s0 is unused.
    """
    if _OPNAME in dve_ops._SUB_OPCODE_FOR_NAME:
        for op in dve_ops.OPS:
            if op.name == _OPNAME:
                return op
    spec = Spec(body=((Src0 * Src1 + DVE_C1) - DVE_C1) * DVE_C2 + Idx,
                accum=maxx, accum_init=Zero, reference=_ref_pack_idx_rmax)
    row = max(dve_ops._SUB_OPCODE_FOR_NAME.values()) + 1
    assert row < 0x20
    shas = {}
    for ver in ("v3", "v4"):
        try:
            s = DveOpSpec(name=_OPNAME, opcode=row, uops=lower(spec, ver=ver),
                          rd1_en=_has_src1(spec))
            shas[ver] = s.sha(ver)
        except Exception:
            pass
    op = DveOp(_OPNAME, spec, subdim=False, uops_sha=shas)
    dve_ops.OPS.append(op)
    dve_ops.CUSTOM_DVE_SPECS[_OPNAME] = spec
    dve_ops._SUB_OPCODE_FOR_NAME[_OPNAME] = row
    return op


register_tt_mul_rmax = register_pack_idx_rmax  # back-compat alias

_OPNAME_MIN = "PACK_IDX_RMIN_ANT"


def _ref_pack_idx_rmin(in0, in1, s0, s1, imm2):
    t = (in0.astype(np.float32) * in1).astype(np.float32)
    q = np.float32(np.float32(t + s1) - s1)
    q = q.reshape(q.shape[0], -1)
    p = np.float32(np.float32(s1) - np.float32(
        q * np.float32(imm2)
        + np.arange(q.shape[-1], dtype=np.float32)[None, :]))
    mx = np.maximum(np.float32(0.0), p.max(axis=-1, keepdims=True))
    return p.reshape(in0.shape), mx.astype(np.float32)


def register_pack_idx_rmin():
    """Min twin of PACK_IDX_RMAX_ANT via inverted packing:
    p[k] = 2^23 - (round(in0*in1)*imm2 + k), accum = max(0, row-max(p))
    = 2^23 - min(packed). Valid candidates stay positive so the free
    Zero accum-init is safe (MaxNeg/C0 inits exceed the 6 carry lanes);
    invalid {weight 200} entries go hugely negative and lose.

    Mines the hardest negative off the SAME staged tensor as the max op:
    the {1,200} weight mask keeps invalid entries out of the min (200*d2
    overflows the exact-packing range, but losers only need to be large,
    not exact), so no second ACT staging pass is needed.
    """
    if _OPNAME_MIN in dve_ops._SUB_OPCODE_FOR_NAME:
        for op in dve_ops.OPS:
            if op.name == _OPNAME_MIN:
                return op
    spec = Spec(body=DVE_C1 - (((Src0 * Src1 + DVE_C1) - DVE_C1) * DVE_C2 + Idx),
                accum=maxx, accum_init=Zero, reference=_ref_pack_idx_rmin)
    row = max(dve_ops._SUB_OPCODE_FOR_NAME.values()) + 1
    assert row < 0x20
    shas = {}
    for ver in ("v3", "v4"):
        try:
            s = DveOpSpec(name=_OPNAME_MIN, opcode=row, uops=lower(spec, ver=ver),
                          rd1_en=_has_src1(spec))
            shas[ver] = s.sha(ver)
        except Exception:
            pass
    op = DveOp(_OPNAME_MIN, spec, subdim=False, uops_sha=shas)
    dve_ops.OPS.append(op)
    dve_ops.CUSTOM_DVE_SPECS[_OPNAME_MIN] = spec
    dve_ops._SUB_OPCODE_FOR_NAME[_OPNAME_MIN] = row
    return op

B, D = 4096, 128
M = 8              # cores
BL = B // M        # 512 anchors per core
P = 128            # partition block
NB = BL // P       # 4 anchor blocks per core
CH = 512           # psum chunk (one bank of f32)
NCH = B // CH      # 8 chunks
EPS = 1e-6
C0 = 32.0
MARGIN = 1.0

F32 = mybir.dt.float32
BF16 = mybir.dt.bfloat16
U8 = mybir.dt.uint8
I8 = mybir.dt.int8
U32 = mybir.dt.uint32
TWO23 = float(2.0 ** 23)
PACK = 4096.0
NEG_K = 2000.0
VTH_P = 32.0 * PACK        # packed validity threshold, pos
VTH_N = 1.0e6              # packed validity threshold, neg (inverted-min side)


def build_nc(debug: bool = False):
    pack_op = register_pack_idx_rmax()
    pack_min = register_pack_idx_rmin()
    nc = bacc.Bacc("TRN2", target_bir_lowering=False, debug=debug)

    eT = nc.dram_tensor("eT", [P, BL], BF16, kind="ExternalInput")      # -4*e_local^T
    gT = nc.dram_tensor("gT", [P, B], BF16, kind="ExternalInput")       # g^T
    cgo = nc.dram_tensor("cgo", [2, B], BF16, kind="ExternalInput")     # [2*cg ; ones]
    ar2 = nc.dram_tensor("ar2", [2, BL], BF16, kind="ExternalInput")    # [ones ; 2*arow]
    el = nc.dram_tensor("el", [P, NB, D], F32, kind="ExternalInput")    # anchor rows f32
    gfull = nc.dram_tensor("gfull", [B, D], F32, kind="ExternalInput")  # gather source
    mp = nc.dram_tensor("mp", [BL, B], U8, kind="ExternalInput")        # pos mask {0,1}
    wn = nc.dram_tensor("wn", [BL, B], U8, kind="ExternalInput")        # neg weights {1,200}

    lossv = nc.dram_tensor("lossv", [P, NB], F32, kind="ExternalOutput")
    vout = nc.dram_tensor("vout", [P, NB], F32, kind="ExternalOutput")

    with tile.TileContext(nc) as tc:
        with (
            tc.tile_pool(name="singles", bufs=1) as singles,
            tc.tile_pool(name="masks", bufs=3) as maskpool,
            tc.tile_pool(name="vscr", bufs=1) as vpool,
            tc.tile_pool(name="stage", bufs=2) as stagepool,
            tc.tile_pool(name="psum", bufs=2, space="PSUM") as psumpool,
            tc.tile_pool(name="sm", bufs=1) as sm,
            tc.tile_pool(name="blk", bufs=2) as blk,
        ):
            HB = B // 2          # 2048: one PSUM half (4 banks)
            HCH = HB // CH       # 4 chunks per half
            HB0 = HB             # first-mask split point
            # block-sliced eT and chunk-sliced gT loads so the first matmul
            # only waits on the slices it actually reads; block-0 masks are
            # interleaved after gT's first half so they land before mining
            eT_s = singles.tile([P, BL], BF16)
            nc.sync.dma_start(eT_s[:, 0:P], eT[:, 0:P])
            gT_s = singles.tile([P, B], BF16)
            for c in range(NCH):
                cs = slice(c * CH, (c + 1) * CH)
                nc.sync.dma_start(gT_s[:, cs], gT[:, cs])
            for b in range(1, NB):
                bs = slice(b * P, (b + 1) * P)
                nc.sync.dma_start(eT_s[:, bs], eT[:, bs])
            cgo_s = singles.tile([2, B], BF16)
            nc.sync.dma_start(cgo_s[:], cgo[:])
            ar2_s = singles.tile([2, BL], BF16)
            nc.sync.dma_start(ar2_s[:], ar2[:])
            eps_b = singles.tile([P, 1], F32)
            nc.vector.memset(eps_b[:], EPS)
            # touch Sqrt/Square/Relu once so ACT's table swap lands in the
            # fill shadow instead of the tail
            warm = singles.tile([P, 1], F32)
            nc.scalar.activation(warm[:], eps_b[:],
                                 mybir.ActivationFunctionType.Square)
            nc.scalar.activation(warm[:], warm[:],
                                 mybir.ActivationFunctionType.Sqrt)
            nc.scalar.activation(warm[:], warm[:],
                                 mybir.ActivationFunctionType.Relu)

            # batched per-anchor state (host pre-arranged contiguous):
            el_all = singles.tile([P, NB, D], F32)
            Mp_all = singles.tile([P, NB], F32)
            Mn_all = singles.tile([P, NB], F32)
            idxp_all = singles.tile([P, NB], U32)
            idxn_all = singles.tile([P, NB], U32)
            p_all = singles.tile([P, NB, D], F32)
            n_all = singles.tile([P, NB, D], F32)
            m4095 = singles.tile([P, 1], U32)
            nc.vector.memset(m4095[:], 4095)

            for b in range(NB):
                rs = b * P
                mp_b = maskpool.tile([P, B], U8, tag="mp")
                nc.sync.dma_start(mp_b[:], mp[rs:rs + P, :])
                wn_b = maskpool.tile([P, B], U8, tag="wn")
                nc.sync.dma_start(wn_b[:], wn[rs:rs + P, :])
                if b == 1:
                    # tail-only data, loaded once the startup rush is over
                    nc.scalar.dma_start(el_all[:], el[:])

                v = vpool.tile([P, B], F32)
                # block 0 splits the first half into small pieces so the first
                # mining op starts as soon as one 512-col PSUM bank is ready
                pieces = [(0, HB), (HB, HB)]
                NPC = len(pieces)
                Ph = blk.tile([P, 2 * 3], F32, tag="Ph")  # [pos x pieces|neg x pieces]
                for pi, (hs, plen) in enumerate(pieces):
                    # grouped by lhsT so LDWEIGHTS isn't reloaded per chunk
                    psum = psumpool.tile([P, HB], F32, tag="psum")
                    pch = plen // CH
                    for c in range(pch):
                        cs = slice(hs + c * CH, hs + (c + 1) * CH)
                        ps = slice(c * CH, (c + 1) * CH)
                        nc.tensor.matmul(
                            psum[:, ps], lhsT=eT_s[:, rs:rs + P],
                            rhs=gT_s[:, cs], start=True, stop=False,
                        )
                    for c in range(pch):
                        cs = slice(hs + c * CH, hs + (c + 1) * CH)
                        ps = slice(c * CH, (c + 1) * CH)
                        nc.tensor.matmul(
                            psum[:, ps], lhsT=ar2_s[:, rs:rs + P],
                            rhs=cgo_s[:, cs], start=False, stop=True,
                        )

                    # ACT copies PSUM to SBUF so the PSUM slot is released
                    # after ~2us for PE's next half (DVE holding PSUM through
                    # both packs serializes PE and costs ~4us/block)
                    hsl = slice(hs, hs + plen)
                    dps = stagepool.tile([P, HB], F32, tag="dps")
                    nc.scalar.activation(
                        dps[:, 0:plen], psum[:, 0:plen],
                        mybir.ActivationFunctionType.Copy)

                    # packed mining (per-anchor bias rode the K=2 rank-1
                    # matmul): one DVE pass per side yields
                    # max(round(value)*4096 + local_idx) per partition
                    nc.vector._custom_dve(
                        pack_op, out=v[:, hsl], in0=dps[:, 0:plen],
                        in1=mp_b[:, hsl], s0=0.0, s1=TWO23, imm2=PACK,
                        accum_out=Ph[:, pi:pi + 1])
                    nc.vector._custom_dve(
                        pack_min, out=v[:, hsl], in0=dps[:, 0:plen],
                        in1=wn_b[:, hsl], s0=0.0, s1=TWO23, imm2=PACK,
                        accum_out=Ph[:, 3 + pi:4 + pi])

                # merge pieces (local idx -> global: +piece offset in idx field)
                for pi, (hs, plen) in enumerate(pieces):
                    if hs == 0:
                        continue
                    nc.vector.tensor_scalar(
                        Ph[:, pi:pi + 1], Ph[:, pi:pi + 1], float(hs),
                        scalar2=None, op0=mybir.AluOpType.add)
                    nc.vector.tensor_scalar(
                        Ph[:, 3 + pi:4 + pi], Ph[:, 3 + pi:4 + pi], float(-hs),
                        scalar2=None, op0=mybir.AluOpType.add)
                nc.vector.tensor_tensor(out=Mp_all[:, b:b + 1], in0=Ph[:, 0:1],
                                        in1=Ph[:, 1:2], op=mybir.AluOpType.max)
                nc.vector.tensor_tensor(out=Mn_all[:, b:b + 1], in0=Ph[:, 3:4],
                                        in1=Ph[:, 4:5], op=mybir.AluOpType.max)
                if NPC > 2:
                    nc.vector.tensor_tensor(
                        out=Mp_all[:, b:b + 1], in0=Mp_all[:, b:b + 1],
                        in1=Ph[:, 2:3], op=mybir.AluOpType.max)
                    nc.vector.tensor_tensor(
                        out=Mn_all[:, b:b + 1], in0=Mn_all[:, b:b + 1],
                        in1=Ph[:, 5:6], op=mybir.AluOpType.max)
                # decode indices: +2^23 pins ulp=1, low 12 mantissa bits = idx
                p23 = blk.tile([P, 2], F32, tag="p23")
                nc.vector.tensor_scalar(p23[:, 0:1], Mp_all[:, b:b + 1], TWO23,
                                        scalar2=None, op0=mybir.AluOpType.add)
                nc.vector.tensor_scalar(p23[:, 1:2], Mn_all[:, b:b + 1], -1.0,
                                        scalar2=None, op0=mybir.AluOpType.mult)
                nc.vector.tensor_scalar(p23[:, 1:2], p23[:, 1:2], 2.0 * TWO23,
                                        scalar2=None, op0=mybir.AluOpType.add)
                nc.vector.tensor_tensor(
                    out=idxp_all[:, b:b + 1], in0=p23[:, 0:1].bitcast(U32),
                    in1=m4095[:, 0:1], op=mybir.AluOpType.bitwise_and)
                nc.vector.tensor_tensor(
                    out=idxn_all[:, b:b + 1], in0=p23[:, 1:2].bitcast(U32),
                    in1=m4095[:, 0:1], op=mybir.AluOpType.bitwise_and)

                nc.gpsimd.indirect_dma_start(
                    out=p_all[:, b, :], out_offset=None, in_=gfull[:],
                    in_offset=bass.IndirectOffsetOnAxis(
                        ap=idxp_all[:, b:b + 1], axis=0),
                )
                nc.gpsimd.indirect_dma_start(
                    out=n_all[:, b, :], out_offset=None, in_=gfull[:],
                    in_offset=bass.IndirectOffsetOnAxis(
                        ap=idxn_all[:, b:b + 1], axis=0),
                )

            # ---- batched tail ----
            # exact f32: ap=||a-p+eps||, an=||a-n+eps||, pn=||p-n+eps||
            # split: blocks [0, NB-1) first (their gathers are long done while
            # block NB-1's gathers are still in flight), then the last block
            # validity first: depends only on the TTR accums, fills the DVE
            # stream while the last block's gathers are in flight
            vp = sm.tile([P, NB], F32)
            vn = sm.tile([P, NB], F32)
            valid = sm.tile([P, NB], F32)
            nc.vector.tensor_scalar(vp[:], Mp_all[:], VTH_P, scalar2=None,
                                    op0=mybir.AluOpType.is_gt)
            nc.vector.tensor_scalar(vn[:], Mn_all[:], VTH_N, scalar2=None,
                                    op0=mybir.AluOpType.is_gt)
            nc.vector.tensor_mul(valid[:], vp[:], vn[:])

            dif = sm.tile([P, NB, D], F32)
            sq = sm.tile([P, NB, D], F32)
            rt2 = sm.tile([P, 3 * NB], F32)   # [ap2 x NB | an2 x NB | pn2 x NB]
            pairs = ((el_all, p_all), (el_all, n_all), (p_all, n_all))
            for lo, hi in ((0, NB - 1), (NB - 1, NB)):
                n = hi - lo
                for k, (x, y) in enumerate(pairs):
                    nc.vector.tensor_sub(dif[:, lo:hi, :], x[:, lo:hi, :],
                                         y[:, lo:hi, :])
                    nc.scalar.activation(sq[:, lo:hi, :], dif[:, lo:hi, :],
                                         mybir.ActivationFunctionType.Square,
                                         bias=eps_b[:, 0:1], scale=1.0)
                    nc.vector.tensor_reduce(
                        out=rt2[:, k * NB + lo:k * NB + hi],
                        in_=sq[:, lo:hi, :],
                        axis=mybir.AxisListType.X, op=mybir.AluOpType.add)
            rt = sm.tile([P, 3 * NB], F32)
            nc.scalar.activation(rt[:], rt2[:], mybir.ActivationFunctionType.Sqrt)

            mn2 = sm.tile([P, NB], F32)
            nc.vector.tensor_tensor(out=mn2[:], in0=rt[:, NB:2 * NB],
                                    in1=rt[:, 2 * NB:3 * NB],
                                    op=mybir.AluOpType.min)
            dff = sm.tile([P, NB], F32)
            nc.vector.tensor_sub(dff[:], rt[:, 0:NB], mn2[:])
            lossb = sm.tile([P, NB], F32)
            nc.scalar.activation(lossb[:], dff[:],
                                 mybir.ActivationFunctionType.Relu,
                                 bias=MARGIN, scale=1.0)
            lout = sm.tile([P, NB], F32)
            nc.vector.tensor_mul(lout[:], lossb[:], valid[:])

            nc.sync.dma_start(lossv[:], lout[:])
            nc.sync.dma_start(vout[:], valid[:])

    nc.finalize()
    return nc


def make_in_maps(embedding, target_idx, positive_idxs, negative_idxs):
    e = np.asarray(embedding, np.float32)
    tid = np.asarray(target_idx, np.int64)
    pos = np.asarray(positive_idxs)
    neg = np.asarray(negative_idxs)

    inv = np.empty(B, np.int64)
    inv[tid] = np.arange(B)
    g = np.ascontiguousarray(e[inv])                       # [B, D] f32

    e64 = e.astype(np.float64)
    g64 = g.astype(np.float64)
    sq_a = (e64 * e64).sum(1)
    s_a = e64.sum(1)
    sq_g = (g64 * g64).sum(1)
    s_g = g64.sum(1)

    gT_bf = np.ascontiguousarray(g.T).astype(ml_dtypes.bfloat16)         # [D, B]
    # doubled pipeline so packed quantization is 0.5 d2-units
    cgo_np = np.ones((2, B), np.float32)
    cgo_np[0] = 2.0 * (sq_g - 2.0 * EPS * s_g)
    cgo_bf = cgo_np.astype(ml_dtypes.bfloat16)
    arow_full = np.asarray(
        2.0 * (sq_a + 2.0 * EPS * s_a + D * EPS * EPS + C0), np.float32)

    in_maps = []
    for m in range(M):
        r = slice(m * BL, (m + 1) * BL)
        # [P, NB(, D)] layouts: block index on the free axis
        el3 = np.ascontiguousarray(
            e[r].reshape(NB, P, D).transpose(1, 0, 2))
        ar2_np = np.ones((2, BL), np.float32)
        ar2_np[1] = arow_full[r]
        in_maps.append({
            "eT": np.ascontiguousarray((-4.0 * e[r].T)).astype(ml_dtypes.bfloat16),
            "gT": gT_bf,
            "cgo": cgo_bf,
            "ar2": ar2_np.astype(ml_dtypes.bfloat16),
            "el": el3,
            "gfull": g,
            "mp": np.ascontiguousarray(pos[r].astype(np.uint8)),
            "wn": np.ascontiguousarray(np.where(neg[r], 1, 200).astype(np.uint8)),
        })
    return in_maps


_NC_CACHE = {}


def kernel(embedding, target_idx, positive_idxs, negative_idxs):
    in_maps = make_in_maps(embedding, target_idx, positive_idxs, negative_idxs)
    if "nc" not in _NC_CACHE:
        _NC_CACHE["nc"] = build_nc(debug=False)
    nc = _NC_CACHE["nc"]
    res = run_bass_kernel_spmd(nc, in_maps, core_ids=list(range(M)))
    total_loss = np.float64(0.0)
    total_valid = np.float64(0.0)
    for r in res.results:
        total_loss += np.asarray(r["lossv"], np.float64).sum()
        total_valid += np.asarray(r["vout"], np.float64).sum()
    return np.float32(total_loss / max(total_valid, 1.0))
